# revision 1
# baseline (speedup 1.0000x reference)
"""GAT (graph attention) message-passing kernel for 8 Trainium2 NeuronCores.

Strategy (degree-sorted, dst-sharded graph parallel):
  - Host: add self loops; compute each node's in-degree; sort nodes by
    degree (desc) and deal them round-robin to the 8 cores so every core
    gets the same degree profile. Each core's 12500 nodes form 98 blocks
    of 128 similar-degree nodes; node <-> SBUF partition, and the node's
    edges occupy free-dim slots 0..deg-1. Block tile count = max degree in
    the block (degree sorting keeps padding small). Host work is pure
    index/permutation preprocessing.
  - Device phase 1 (per core, own nodes):
    h_ext = x_shard @ [W | W@a_src | W@a_dst] -> node table row
    [h (128 f32) | alpha_src (4 f32)] (528B) + local alpha_dst[NLOC, 4].
  - AllGather the node table so each core can gather any source row.
  - Device phase 3, per dst block: alpha_dst block = plain [128,4] DMA
    slice (node==partition!). Per 128-edge tile column: one [128,1]
    indirect DMA gathers h_ext[src] rows; then
      t = alpha_s + alpha_d + mask   (mask = -1e30 on padding slots)
      p = exp(lrelu(t)) = max(exp(t), exp(0.2 t))     (ACT engine)
      rhs = [p*h | p]; psum[128, 132] += I.T @ rhs     (PE accumulate)
    Block epilogue: out = tanh(psum[:, :128] / psum[:, 128:132][head] + bias),
    written to permuted rows; host inverts the permutation at the end.

Softmax max-subtraction is dropped: logits are O(+-5), exp is safe in fp32
and softmax is shift-invariant, so results match the reference to fp error.
"""

import os
import numpy as np

import concourse.bacc as bacc
import concourse.bass as bass
import concourse.mybir as mybir
import concourse.tile as tile

NCORES = 8
P = 128          # partitions / block size / h channels
H = 4            # heads
C = 32           # channels per head
TBL = P + H      # 132 table row floats
GRP = 8          # DVE batch, in tiles

F32 = mybir.dt.float32
F16 = mybir.dt.float16
I32 = mybir.dt.int32

NEG = -1.0e30


def build_program(N, NLOC, IN_DIM, Tb):
    """One SPMD program shared by all cores. Tb[b] = tiles for dst block b
    (uniform across cores)."""
    NB = (NLOC + P - 1) // P
    last_nn = NLOC - (NB - 1) * P
    KT = IN_DIM // P
    T_tot = int(np.sum(Tb))
    tau0 = np.zeros(NB, dtype=np.int64)
    tau0[1:] = np.cumsum(Tb)[:-1]

    nc = bacc.Bacc("TRN2", target_bir_lowering=False, num_devices=NCORES)

    xT = nc.dram_tensor("xT", [IN_DIM, NLOC], F32, kind="ExternalInput")
    Wt = nc.dram_tensor("W", [IN_DIM, P], F32, kind="ExternalInput")
    asrc_b = nc.dram_tensor("asrc_b", [P, P], F32, kind="ExternalInput")
    adst_b = nc.dram_tensor("adst_b", [P, P], F32, kind="ExternalInput")
    bias_b = nc.dram_tensor("bias_b", [P, P], F32, kind="ExternalInput")
    ident_in = nc.dram_tensor("ident", [P, P], F32, kind="ExternalInput")
    gidx_in = nc.dram_tensor("gidx", [P, T_tot], I32, kind="ExternalInput")
    mask_in = nc.dram_tensor("mask", [P, T_tot], F32, kind="ExternalInput")

    out_d = nc.dram_tensor("out", [NLOC, P], F32, kind="ExternalOutput")

    tbl_own = nc.dram_tensor("tbl_own", [NLOC, TBL], F32, kind="Internal")
    tbl_full = nc.dram_tensor("tbl_full", [N, TBL], F32, kind="Internal",
                              addr_space="Shared")
    ad_dram = nc.dram_tensor("ad_tbl", [NLOC, H], F32, kind="Internal")

    with tile.TileContext(nc) as tc:
        with tc.tile_pool(name="const", bufs=1) as cpool, \
             tc.tile_pool(name="p1", bufs=3) as p1pool, \
             tc.tile_pool(name="gat", bufs=6) as gpool, \
             tc.tile_pool(name="wrk", bufs=3) as wpool, \
             tc.tile_pool(name="epi", bufs=3) as epool, \
             tc.tile_pool(name="ps", bufs=3, space="PSUM") as ppool, \
             tc.tile_pool(name="ps1", bufs=2, space="PSUM") as p1ps:

            # ---- Phase 0: constants and streams ----
            ident_sb = cpool.tile([P, P], F32)
            nc.sync.dma_start(out=ident_sb[:], in_=ident_in[:, :])
            asrc_sb = cpool.tile([P, P], F32)
            nc.sync.dma_start(out=asrc_sb[:], in_=asrc_b[:, :])
            adst_sb = cpool.tile([P, P], F32)
            nc.sync.dma_start(out=adst_sb[:], in_=adst_b[:, :])
            bias_sb = cpool.tile([P, P], F32)
            nc.sync.dma_start(out=bias_sb[:], in_=bias_b[:, :])
            gidx_sb = cpool.tile([P, T_tot], I32)
            nc.sync.dma_start(out=gidx_sb[:], in_=gidx_in[:, :])
            mask_sb = cpool.tile([P, T_tot], F32)
            nc.sync.dma_start(out=mask_sb[:], in_=mask_in[:, :])

            # Extended weights [W | W@a_src | W@a_dst] per 128-row k-tile.
            W_sb = cpool.tile([P, KT, TBL + H], F32)
            scr = cpool.tile([P, P], F32)
            for kt in range(KT):
                nc.sync.dma_start(out=W_sb[:, kt, 0:P],
                                  in_=Wt[kt * P:(kt + 1) * P, :])
            for kt in range(KT):
                for j, ab in ((0, asrc_sb), (1, adst_sb)):
                    nc.vector.tensor_tensor(
                        out=scr[:], in0=W_sb[:, kt, 0:P], in1=ab[:],
                        op=mybir.AluOpType.mult)
                    nc.vector.tensor_reduce(
                        out=W_sb[:, kt, P + j * H:P + (j + 1) * H],
                        in_=scr[:].rearrange("p (h c) -> p h c", h=H),
                        axis=mybir.AxisListType.X,
                        op=mybir.AluOpType.add)

            # ---- Phase 1: node table + alpha_dst for own shard ----
            NT = (NLOC + P - 1) // P
            XB = 4                       # n-tiles loaded per DMA
            xt = None
            for nt in range(NT):
                nn = min(P, NLOC - nt * P)
                if nt % XB == 0:
                    bw = min(XB * P, NLOC - nt * P)
                    xt = p1pool.tile([P, KT, XB * P], F32, tag="xt")
                    for kt in range(KT):
                        nc.sync.dma_start(
                            out=xt[:, kt, :bw],
                            in_=xT[kt * P:(kt + 1) * P, nt * P:nt * P + bw])
                o = (nt % XB) * P
                ps1 = p1ps.tile([P, TBL + H], F32)
                for kt in range(KT):
                    nc.tensor.matmul(out=ps1[:nn, :],
                                     lhsT=xt[:, kt, o:o + nn],
                                     rhs=W_sb[:, kt, :],
                                     start=(kt == 0), stop=(kt == KT - 1))
                trow = p1pool.tile([P, TBL], F32, tag="trow")
                nc.vector.tensor_copy(out=trow[:nn, :], in_=ps1[:nn, 0:TBL])
                nc.sync.dma_start(out=tbl_own[nt * P:nt * P + nn, :],
                                  in_=trow[:nn, :])
                adrow = p1pool.tile([P, H], F32, tag="adrow")
                nc.vector.tensor_copy(out=adrow[:nn, :],
                                      in_=ps1[:nn, TBL:TBL + H])
                nc.sync.dma_start(out=ad_dram[nt * P:nt * P + nn, :],
                                  in_=adrow[:nn, :])

            tc.strict_bb_all_engine_barrier()

            # ---- Phase 2: replicate node table ----
            nc.gpsimd.collective_compute(
                kind="AllGather",
                op=mybir.AluOpType.bypass,
                replica_groups=[list(range(NCORES))],
                ins=[tbl_own[:, :]],
                outs=[tbl_full[:, :]],
            )

            tc.strict_bb_all_engine_barrier()

            # ---- Phase 3: gather / attention / accumulate per dst block ----
            for b in range(NB):
                nn = last_nn if b == NB - 1 else P
                t0 = int(tau0[b])
                tb = int(Tb[b])
                adb = epool.tile([P, H], F32, tag="adb")
                nc.sync.dma_start(out=adb[:nn, :],
                                  in_=ad_dram[b * P:b * P + nn, :])
                pt = ppool.tile([P, TBL], F32, name="pblk", tag="pblk")
                for g0 in range(0, tb, GRP):
                    gw = min(GRP, tb - g0)
                    gbuf = gpool.tile([P, GRP, TBL], F32, tag="gbuf")
                    for i in range(gw):
                        nc.gpsimd.indirect_dma_start(
                            out=gbuf[:, i, :], out_offset=None,
                            in_=tbl_full[:, :],
                            in_offset=bass.IndirectOffsetOnAxis(
                                ap=gidx_sb[:, t0 + g0 + i:t0 + g0 + i + 1],
                                axis=0),
                        )
                    # t = alpha_s + alpha_d + mask
                    tsc = wpool.tile([P, GRP, H], F32, tag="tsc")
                    nc.vector.tensor_tensor(
                        out=tsc[:, :gw, :],
                        in0=gbuf[:, :gw, P:TBL],
                        in1=adb[:, None, :].broadcast_to([P, gw, H]),
                        op=mybir.AluOpType.add)
                    nc.vector.tensor_tensor(
                        out=tsc[:, :gw, :],
                        in0=tsc[:, :gw, :],
                        in1=mask_sb[:, t0 + g0:t0 + g0 + gw][:, :, None]
                            .broadcast_to([P, gw, H]),
                        op=mybir.AluOpType.add)
                    # p = max(exp(t), exp(0.2 t))
                    e1 = wpool.tile([P, GRP, H], F32, tag="e1")
                    nc.scalar.activation(
                        out=e1[:, :gw, :], in_=tsc[:, :gw, :],
                        func=mybir.ActivationFunctionType.Exp)
                    e2 = wpool.tile([P, GRP, H], F32, tag="e2")
                    nc.scalar.activation(
                        out=e2[:, :gw, :], in_=tsc[:, :gw, :],
                        func=mybir.ActivationFunctionType.Exp, scale=0.2)
                    nc.vector.tensor_tensor(
                        out=gbuf[:, :gw, P:TBL], in0=e1[:, :gw, :],
                        in1=e2[:, :gw, :], op=mybir.AluOpType.max)
                    # h *= p (per head), in place -> rhs = [p*h | p]
                    nc.vector.tensor_tensor(
                        out=gbuf[:, :gw, 0:P].rearrange(
                            "p t (h c) -> p t h c", h=H),
                        in0=gbuf[:, :gw, 0:P].rearrange(
                            "p t (h c) -> p t h c", h=H),
                        in1=gbuf[:, :gw, P:TBL][:, :, :, None].broadcast_to(
                            [P, gw, H, C]),
                        op=mybir.AluOpType.mult)
                    for i in range(gw):
                        nc.tensor.matmul(out=pt[:],
                                         lhsT=ident_sb[:],
                                         rhs=gbuf[:, i, :],
                                         start=(g0 + i == 0),
                                         stop=(g0 + i == tb - 1))
                # epilogue
                rcp = epool.tile([P, H], F32, tag="rcp")
                nc.vector.reciprocal(rcp[:nn, :], pt[:nn, P:TBL])
                osb = epool.tile([P, P], F32, tag="osb")
                nc.vector.tensor_tensor(
                    out=osb[:nn, :].rearrange("p (h c) -> p h c", h=H),
                    in0=pt[:nn, 0:P].rearrange("p (h c) -> p h c", h=H),
                    in1=rcp[:nn, :, None].broadcast_to([nn, H, C]),
                    op=mybir.AluOpType.mult)
                nc.vector.tensor_tensor(
                    out=osb[:nn, :], in0=osb[:nn, :], in1=bias_sb[:nn, :],
                    op=mybir.AluOpType.add)
                fin = epool.tile([P, P], F32, tag="fin")
                nc.scalar.activation(
                    out=fin[:nn, :], in_=osb[:nn, :],
                    func=mybir.ActivationFunctionType.Tanh)
                nc.sync.dma_start(out=out_d[b * P:b * P + nn, :],
                                  in_=fin[:nn, :])
    nc.finalize()
    return nc


def host_prepare(x, W, a_src, a_dst, bias, edge_index):
    """Index/permutation preprocessing only - no float math on node data."""
    N, IN_DIM = x.shape
    NLOC = N // NCORES
    NB = (NLOC + P - 1) // P

    ei = np.asarray(edge_index)
    loops = np.arange(N, dtype=np.int64)
    src = np.concatenate([ei[0].astype(np.int64), loops])
    dst = np.concatenate([ei[1].astype(np.int64), loops])

    deg = np.bincount(dst, minlength=N)
    rank = np.argsort(-deg, kind="stable")      # rank r -> node
    rank_inv = np.empty(N, dtype=np.int64)      # node -> rank
    rank_inv[rank] = np.arange(N)

    r_of_dst = rank_inv[dst]
    core_of = r_of_dst % NCORES
    loc_of = r_of_dst // NCORES
    tblpos = (rank_inv % NCORES) * NLOC + rank_inv // NCORES  # node -> row

    deg_by_rank = deg[rank]                      # descending
    Tb = np.zeros(NB, dtype=np.int64)
    for b in range(NB):
        lo = b * P * NCORES
        hi = min(N, (b + 1) * P * NCORES)
        Tb[b] = max(1, deg_by_rank[lo:hi].max())
    tau0 = np.zeros(NB, dtype=np.int64)
    tau0[1:] = np.cumsum(Tb)[:-1]
    T_tot = int(np.sum(Tb))

    ident = np.eye(P, dtype=np.float32)
    asrc_b = np.tile(np.asarray(a_src, np.float32).reshape(1, -1), (P, 1))
    adst_b = np.tile(np.asarray(a_dst, np.float32).reshape(1, -1), (P, 1))
    bias_b = np.tile(np.asarray(bias, np.float32).reshape(1, -1), (P, 1))
    Wf = np.ascontiguousarray(np.asarray(W, np.float32))
    xf = np.asarray(x, np.float32)

    in_maps = []
    for k in range(NCORES):
        sel = np.flatnonzero(core_of == k)
        l_k = loc_of[sel]
        s_k = src[sel]
        order = np.argsort(l_k, kind="stable")
        l_k, s_k = l_k[order], s_k[order]
        starts = np.zeros(NLOC, dtype=np.int64)
        cnts = np.bincount(l_k, minlength=NLOC)
        starts[1:] = np.cumsum(cnts)[:-1]
        t = np.arange(len(l_k)) - starts[l_k]    # edge slot within node
        blk = l_k // P
        pp = l_k % P
        col = tau0[blk] + t

        gidx = np.zeros((P, T_tot), dtype=np.int32)
        mask = np.full((P, T_tot), NEG, dtype=np.float32)
        gidx[pp, col] = tblpos[s_k]
        mask[pp, col] = 0.0

        own_nodes = rank[np.arange(NLOC) * NCORES + k]
        xT_k = np.ascontiguousarray(xf[own_nodes].T)
        in_maps.append({
            "xT": xT_k, "W": Wf, "asrc_b": asrc_b, "adst_b": adst_b,
            "bias_b": bias_b, "ident": ident, "gidx": gidx, "mask": mask,
        })

    cfg = dict(N=N, NLOC=NLOC, IN_DIM=IN_DIM)
    meta = dict(Tb=Tb.tolist(), rank=rank)
    return cfg, meta, in_maps


def assemble_output(results, N, rank):
    NLOC = N // NCORES
    out = np.empty((N, P), np.float32)
    for k in range(NCORES):
        own_nodes = rank[np.arange(NLOC) * NCORES + k]
        out[own_nodes] = results[k]["out"]
    return out


LAST_RESULTS = None


def kernel(x, W, a_src, a_dst, bias, edge_index):
    global LAST_RESULTS
    from concourse.bass_utils import run_bass_kernel_spmd

    cfg, meta, in_maps = host_prepare(x, W, a_src, a_dst, bias, edge_index)
    nc = build_program(cfg["N"], cfg["NLOC"], cfg["IN_DIM"], meta["Tb"])
    res = run_bass_kernel_spmd(
        nc, in_maps, core_ids=list(range(NCORES)),
        trace=os.environ.get("GAT_TRACE", "0") == "1")
    LAST_RESULTS = res
    return assemble_output(res.results, cfg["N"], meta["rank"])

